# revision 53
# baseline (speedup 1.0000x reference)
"""GQA kernel for Trainium2, 8-core tensor-parallel over KV groups.

Model: HIDDEN=2048, HEADS=32, GROUPS=8, HEAD_DIM=64, SEQ=2048, BSZ=1.
Core g owns KV group g: its 4 query heads (Wq cols), Wk/Wv col slice,
and Wo row slice. Each core computes a full [SEQ, HIDDEN] partial of the
output projection in bf16; the host sums the 8 partials in fp32 and
adds bo.

Device-side layout: activations transposed (hidden/head_dim on SBUF
partitions), all matmul operands bf16 (same PE throughput as fp32r but
half the DMA/SBUF and no small-tile penalty). Scores are computed as
S^T [t, s] tiles; softmax needs no max-subtraction (|scores| <~ 8); the
softmax denominator comes free from an appended ones-column on V in the
P^T @ V_aug matmul. The causal mask for diagonal tiles is accumulated
into PSUM by an extra identity x mask matmul on the PE (keeps the
score->exp chain off the vector engine). The softmax-denominator
broadcast reuses the unused rows 64:128 of the AV PSUM tile. PSUM->SBUF
output drains alternate between the GpSimd and Vector engines.
"""

import sys
import numpy as np

for _p in ("/opt/trn_rl_repo", "/root/.axon_site/_ro/trn_rl_repo"):
    if _p not in sys.path:
        sys.path.insert(0, _p)

import concourse.bass as bass
import concourse.tile as tile
from concourse import mybir
from concourse.bass_utils import run_bass_kernel_spmd

S = 2048          # sequence length
H = 2048          # hidden
G = 8             # kv groups == cores
R = 4             # query heads per group
D = 64            # head dim
HDG = R * D       # 256 per-core q width
NT = S // 128     # 16 t-tiles
NC = S // 512     # 4 s-chunks
NH = H // 128     # 16 h-tiles
F32 = mybir.dt.float32
BF16 = mybir.dt.bfloat16
FP8 = mybir.dt.float8e4

_PROGRAM_CACHE = {}

# fp8e4m3 P/V was measured at ~3.6% attention-output error on this data
# (both signal and quantization noise scale with 1/sqrt(eff_count), so
# averaging buys nothing) — too lossy for the 2e-2 gate. Keep bf16.
USE_FP8 = False


_WAIT_LIMITS = {}


def _split_excess_waits(nc, default_max=1):
    """walrus structs support a limited number of sem waits per
    instruction (TPB_CTRL: 4, matmul's S3_LW: 1). Move the excess onto
    NoOp carriers inserted just before, on the same engine."""
    idx = 0
    for fn in nc.m.functions:
        for blk in fn.blocks:
            insts = list(blk.instructions)
            out = []
            changed = False
            for inst in insts:
                max_waits = _WAIT_LIMITS.get(type(inst).__name__, default_max)
                si = inst.sync_info
                if si is not None and si.on_wait and len(si.on_wait) > max_waits:
                    waits = list(si.on_wait)
                    head, keep = waits[:-max_waits], waits[-max_waits:]
                    while head:
                        chunk, head = head[:max_waits], head[max_waits:]
                        nop = mybir.InstNoOp(
                            name=f"waitsplit-{idx}",
                            sync_info=mybir.SyncInfo(on_wait=chunk, on_update=[]),
                            engine=inst.engine,
                            bass_nofuse=True,
                        )
                        idx += 1
                        nc.register_instruction(nop)
                        out.append(nop)
                    si.on_wait = keep
                    inst.sync_info = si
                    changed = True
                out.append(inst)
            if changed:
                blk.instructions = out


def _build(causal: bool):
    """Build the SPMD program (same for all cores; data differs)."""
    nc = bass.Bass(trn_type="TRN2", target_bir_lowering=False, debug=False)

    xT = nc.dram_tensor("xT", [H, S], BF16, kind="ExternalInput").ap()
    wq = nc.dram_tensor("wq", [128, NH, HDG], BF16, kind="ExternalInput").ap()
    bq = nc.dram_tensor("bq", [128, 2], F32, kind="ExternalInput").ap()
    wkv = nc.dram_tensor("wkv", [128, NH, 2 * D], BF16, kind="ExternalInput").ap()
    bkv = nc.dram_tensor("bkv", [128, 1], F32, kind="ExternalInput").ap()
    wo = nc.dram_tensor("wo", [128, 2, H], BF16, kind="ExternalInput").ap()
    mblk = nc.dram_tensor("mblk", [128, 128], BF16, kind="ExternalInput").ap()
    identB = nc.dram_tensor("identB", [128, 128], BF16, kind="ExternalInput").ap()
    mneg = nc.dram_tensor("mneg", [128, 128], BF16, kind="ExternalInput").ap()
    ident64 = nc.dram_tensor("ident64", [128, 64], F32, kind="ExternalInput").ap()
    ones64 = nc.dram_tensor("ones64", [1, 64], BF16, kind="ExternalInput").ap()
    if not causal:
        mfull = nc.dram_tensor("mfull", [S, S], BF16, kind="ExternalInput").ap()
    partial = nc.dram_tensor("partial", [S, H], BF16, kind="ExternalOutput").ap()

    with tile.TileContext(nc) as tc:
        with (
            tc.tile_pool(name="wpool", bufs=1) as wpool,
            tc.tile_pool(name="big", bufs=1) as big,
            tc.tile_pool(name="xp", bufs=6) as xp,
            tc.tile_pool(name="ptp", bufs=4) as ptp,
            tc.tile_pool(name="rbp", bufs=3) as rbp,
            tc.tile_pool(name="opp", bufs=3) as opp,
            tc.tile_pool(name="mfp", bufs=8) as mfp,
            tc.tile_pool(name="psp", bufs=1, space="PSUM") as psp,
        ):
            # ---- warm-up: the cost model charges a ~3us PE p-state ramp
            # from first use and a 1283ns activation-table load on the
            # first exp; burn both during the DMA-bound start idle ----
            warm_s = wpool.tile([1, 512], BF16)
            nc.vector.memset(warm_s, 0.0)
            warmo_s = wpool.tile([1, 1], BF16)
            nc.scalar.activation(warmo_s, warm_s[0:1, 0:1],
                                 mybir.ActivationFunctionType.Exp)
            pwarm = psp.tile([128, 512], F32, tag="po", bufs=2)
            for _ in range(6):
                nc.tensor.matmul(pwarm, warm_s[0:1, 0:128], warm_s,
                                 start=True, stop=True)

            # ---- resident weights / constants ----
            wq_s = wpool.tile([128, NH, HDG], BF16)
            nc.sync.dma_start(out=wq_s[:, 0:8, :], in_=wq[:, 0:8, :])

            # ---- persistent activations ----
            qT_s = big.tile([128, 2, S], BF16)      # q^T, head-pair major
            k2_s = big.tile([128, S], BF16)         # k^T in both halves
            vT_s = big.tile([128, S], F32)          # v^T in rows 64:128
            vaug_s = big.tile([128, NT, D + 1], BF16)  # v natural + ones col
            vaug8_s = big.tile([128, NT // 2, 2, 80], FP8)  # fp8 t-tile pairs
            attn_s = big.tile([128, 2, S], BF16)    # normalized attn out^T

            bq_s = wpool.tile([128, 2], F32)
            wkv_s = wpool.tile([128, NH, 2 * D], BF16)
            bkv_s = wpool.tile([128, 1], F32)
            ones_s = wpool.tile([1, 64], BF16)
            ident64_s = wpool.tile([128, 64], F32)
            wo_s = wpool.tile([128, 2, H], BF16)
            mblk_s = wpool.tile([128, 128], BF16)
            mneg_s = wpool.tile([128, 128], BF16)
            shift_s = wpool.tile([128, 1], F32)
            identB_s = wpool.tile([128, 128], BF16)

            def proj_x(c, first=False):
                cs = slice(c * 512, (c + 1) * 512)
                xv = []
                if first:
                    # quarter-granularity loads so the first matmuls can
                    # start as early as possible
                    for ib in range(4):
                        xt = xp.tile([128, 4, 512], BF16, name=f"xt0q_{ib}",
                                     tag="xtq", bufs=4)
                        nc.sync.dma_start(
                            out=xt,
                            in_=xT[ib * 512:(ib + 1) * 512, cs]
                            .rearrange("(i p) s -> p i s", p=128))
                        xv.extend(xt[:, i4, :] for i4 in range(4))
                        if ib == 0:
                            nc.sync.dma_start(out=wq_s[:, 8:16, :],
                                              in_=wq[:, 8:16, :])
                else:
                    for ib in range(2):
                        xt = xp.tile([128, 8, 512], BF16, name=f"xt{c}_{ib}",
                                     tag="xt")
                        nc.sync.dma_start(
                            out=xt,
                            in_=xT[ib * 1024:(ib + 1) * 1024, cs]
                            .rearrange("(i p) s -> p i s", p=128))
                        xv.extend(xt[:, i8, :] for i8 in range(8))
                if first:
                    nc.sync.dma_start(out=bq_s, in_=bq)
                    nc.sync.dma_start(out=wkv_s, in_=wkv)
                    nc.sync.dma_start(out=bkv_s, in_=bkv)
                    nc.sync.dma_start(out=ones_s, in_=ones64)
                    nc.sync.dma_start(out=ident64_s, in_=ident64)
                    nc.vector.memset(vaug_s[:, :, 64], 1.0)
                    if USE_FP8:
                        nc.vector.memset(vaug8_s[:, :, :, 64], 1.0)
                        nc.vector.memset(shift_s, -5.0)
                return xv

            def proj_q(c, xts, j):
                cs = slice(c * 512, (c + 1) * 512)
                psq = psp.tile([128, 512], F32, tag="po", bufs=2)
                for i in range(NH):
                    nc.tensor.matmul(
                        psq, wq_s[:, i, j * 128:(j + 1) * 128],
                        xts[i],
                        start=(i == 0), stop=(i == NH - 1))
                nc.scalar.activation(qT_s[:, j, cs], psq,
                                     mybir.ActivationFunctionType.Identity,
                                     bias=bq_s[:, j:j + 1])

            def proj_kv(c, xts):
                cs = slice(c * 512, (c + 1) * 512)
                pskv = psp.tile([128, 512], F32, tag="po", bufs=2)
                for i in range(NH):
                    nc.tensor.matmul(pskv, wkv_s[:, i, :],
                                     xts[i],
                                     start=(i == 0), stop=(i == NH - 1))
                nc.vector.tensor_scalar_add(k2_s[0:64, cs], pskv[0:64, :],
                                            bkv_s[0:64, :])
                nc.vector.tensor_scalar_add(k2_s[64:128, cs], pskv[0:64, :],
                                            bkv_s[0:64, :])
                nc.vector.tensor_scalar_add(vT_s[64:128, cs], pskv[64:128, :],
                                            bkv_s[64:128, :])
                # v natural layout for the AV matmul, via PE transpose (f32)
                for t in range(4 * c, 4 * c + 4):
                    pst = psp.tile([128, 512], F32, tag="po", bufs=2)
                    nc.tensor.transpose(
                        pst[0:128, 0:64],
                        vT_s[64:128, t * 128:(t + 1) * 128],
                        ident64_s[64:128, :])
                    if not USE_FP8 or c == 0 or not causal:
                        nc.vector.tensor_copy(vaug_s[:, t, 0:64],
                                              pst[0:128, 0:64])
                    if USE_FP8 and causal:
                        nc.vector.tensor_copy(
                            vaug8_s[:, t // 2, t % 2, 0:64],
                            pst[0:128, 0:64])

            def attn_pre(c):
                if causal:
                    return None
                cs = slice(c * 512, (c + 1) * 512)
                mf_tiles = []
                for t2 in range(0, NT, 2):
                    mt = mfp.tile([128, 2, 512], BF16, tag="mf")
                    nc.sync.dma_start(
                        out=mt,
                        in_=mfull[t2 * 128:(t2 + 2) * 128, cs]
                        .rearrange("(w p) s -> p w s", p=128))
                    mf_tiles.append(mt)
                return mf_tiles

            def attn_head(c, h, mf_tiles):
                cs = slice(c * 512, (c + 1) * 512)
                n_t = 4 * (c + 1) if causal else NT
                fp8 = USE_FP8 and causal and c > 0
                hp, jj = h % 2, h // 2
                hsl = slice(hp * 64, hp * 64 + 64)
                av = psp.tile([128, 512], F32, tag="av", bufs=2)
                for w in range(n_t // 2):
                    sc = psp.tile([128, 2, 512], F32, tag="sc", bufs=2)
                    if fp8:
                        pt = ptp.tile([128, 2, 512], FP8, tag="pt8",
                                      bufs=12)
                    else:
                        pt = ptp.tile([128, 2, 512], BF16, tag="pt", bufs=8)
                    off0 = 0
                    for k in range(2):
                        t = 2 * w + k
                        off = max(0, t * 128 - c * 512) if causal else 0
                        if k == 0:
                            off0 = off
                        diag = causal and t >= 4 * c
                        if diag:
                            if fp8 and k == 1 and off > off0:
                                # mask tile-b's sub-diagonal garbage so
                                # the paired DoubleRow AV reads exp()=0
                                nc.tensor.matmul(
                                    sc[:, 1, off0:off],
                                    identB_s, mneg_s,
                                    start=True, stop=True)
                            nc.tensor.matmul(
                                sc[:, k, off:512],
                                k2_s[hsl, t * 128:(t + 1) * 128],
                                qT_s[hsl, jj,
                                     c * 512 + off:(c + 1) * 512],
                                start=True, stop=True)
                            nc.vector.tensor_add(
                                sc[:, k, off:off + 128],
                                sc[:, k, off:off + 128], mblk_s)
                        else:
                            nc.tensor.matmul(
                                sc[:, k, :],
                                k2_s[hsl, t * 128:(t + 1) * 128],
                                qT_s[hsl, jj, cs],
                                start=True, stop=True)
                            if not causal:
                                nc.vector.tensor_add(
                                    sc[:, k, :], sc[:, k, :],
                                    mf_tiles[w][:, k, :])
                    if fp8:
                        # exp(score - 5): shifts the fp8 range; the
                        # shift cancels exactly in the softmax ratio
                        nc.scalar.activation(
                            pt[:, :, off0:512], sc[:, :, off0:512],
                            mybir.ActivationFunctionType.Exp, bias=shift_s)
                        nc.tensor.matmul(
                            av[0:65, off0:512],
                            vaug8_s[:, w, :, 0:65],
                            pt[:, :, off0:512],
                            perf_mode=mybir.MatmulPerfMode.DoubleRow,
                            start=(w == 0), stop=(w == n_t // 2 - 1))
                    else:
                        nc.scalar.activation(
                            pt[:, :, off0:512], sc[:, :, off0:512],
                            mybir.ActivationFunctionType.Exp)
                        for k in range(2):
                            t = 2 * w + k
                            off = (max(0, t * 128 - c * 512)
                                   if causal else 0)
                            nc.tensor.matmul(
                                av[0:65, off:512], vaug_s[:, t, :],
                                pt[:, k, off:512],
                                start=(t == 0), stop=(t == n_t - 1))
                # normalize: out^T[d, s] * (1 / rowsum[s]); the rowsum
                # broadcast reuses av rows 64:128 (row 64 is rewritten
                # with the same value it held).
                rs1 = rbp.tile([1, 512], BF16, tag="rs1")
                if causal and c == 3:
                    nc.scalar.activation(rs1, av[64:65, :],
                                         mybir.ActivationFunctionType.Copy)
                else:
                    nc.vector.tensor_copy(rs1, av[64:65, :])
                nc.tensor.matmul(av[64:128, :], ones_s, rs1,
                                 start=True, stop=True)
                rb = rbp.tile([64, 512], F32, tag="rb")
                nc.vector.reciprocal(rb, av[64:128, :])
                nc.vector.tensor_mul(attn_s[hsl, jj, cs], av[0:64, :], rb)

            def outproj(c, sts):
                last = c == NC - 1
                for st_ in sts:
                    s0 = c * 512 + st_ * 128
                    op_s = opp.tile([128, H], BF16, tag="op")
                    for n in range(4):
                        # the last chunk's drains also rotate through the
                        # by-then-idle attention PSUM slots
                        tag = "av" if last and n % 2 else "po"
                        po = psp.tile([128, 512], F32, tag=tag, bufs=2)
                        for j in range(2):
                            nc.tensor.matmul(
                                po, attn_s[:, j, s0:s0 + 128],
                                wo_s[:, j, n * 512:(n + 1) * 512],
                                start=(j == 0), stop=(j == 1))
                        # GPSIMD cannot read PSUM; drain on DVE, with the
                        # (by-then idle) Act engine helping late chunks
                        if c >= 2 and (c == 2 or n % 2 == 0):
                            nc.scalar.activation(
                                op_s[:, n * 512:(n + 1) * 512], po,
                                mybir.ActivationFunctionType.Copy)
                        else:
                            nc.vector.tensor_copy(
                                op_s[:, n * 512:(n + 1) * 512], po)
                        if last and n % 2:
                            nc.sync.dma_start(
                                out=partial[s0:s0 + 128,
                                            (n - 1) * 512:(n + 1) * 512],
                                in_=op_s[:, (n - 1) * 512:(n + 1) * 512])
                    if not last:
                        nc.sync.dma_start(out=partial[s0:s0 + 128, :],
                                          in_=op_s)

            # ---- emission: interleave proj passes / outproj tiles between
            # attention heads so the scheduler always has both spine work
            # (feeding Act) and filler work (keeping PE busy) ----
            xts0 = proj_x(0, first=True)
            nc.sync.dma_start(out=mblk_s, in_=mblk)
            nc.sync.dma_start(out=identB_s, in_=identB)
            nc.sync.dma_start(out=mneg_s, in_=mneg)
            proj_q(0, xts0, 0)
            proj_q(0, xts0, 1)
            proj_kv(0, xts0)
            nc.sync.dma_start(out=wo_s, in_=wo)
            xts1 = proj_x(1)
            mf = attn_pre(0)
            attn_head(0, 0, mf)
            proj_q(1, xts1, 0)
            attn_head(0, 1, mf)
            attn_head(0, 2, mf)
            proj_q(1, xts1, 1)
            attn_head(0, 3, mf)
            proj_kv(1, xts1)
            xts2 = proj_x(2)
            mf = attn_pre(1)
            attn_head(1, 0, mf)
            attn_head(1, 1, mf)
            attn_head(1, 2, mf)
            attn_head(1, 3, mf)
            proj_q(2, xts2, 0)
            proj_q(2, xts2, 1)
            proj_kv(2, xts2)
            xts3 = proj_x(3)
            mf = attn_pre(2)
            attn_head(2, 0, mf)
            attn_head(2, 1, mf)
            attn_head(2, 2, mf)
            attn_head(2, 3, mf)
            proj_q(3, xts3, 0)
            proj_q(3, xts3, 1)
            proj_kv(3, xts3)
            mf = attn_pre(3)
            attn_head(3, 0, mf)
            attn_head(3, 1, mf)
            attn_head(3, 2, mf)
            attn_head(3, 3, mf)
            # outprojs last: lowest priority pure-filler; each becomes
            # ready as soon as its chunk's attention completes
            outproj(0, [0, 1, 2, 3])
            outproj(1, [0, 1, 2, 3])
            outproj(2, [0, 1, 2, 3])
            outproj(3, [0, 1, 2, 3])

    _split_excess_waits(nc)
    return nc


def _get_program(causal: bool):
    if causal not in _PROGRAM_CACHE:
        _PROGRAM_CACHE[causal] = _build(causal)
    return _PROGRAM_CACHE[causal]


def kernel(x, causal_mask, Wq, bq, Wk, bk, Wv, bv, Wo, bo, _trace=False):
    import ml_dtypes
    bf16 = ml_dtypes.bfloat16

    x = np.asarray(x, dtype=np.float32)
    causal_mask = np.asarray(causal_mask, dtype=np.float32)
    Wq = np.asarray(Wq, dtype=np.float32)
    bq = np.asarray(bq, dtype=np.float32)
    Wk = np.asarray(Wk, dtype=np.float32)
    bk = np.asarray(bk, dtype=np.float32)
    Wv = np.asarray(Wv, dtype=np.float32)
    bv = np.asarray(bv, dtype=np.float32)
    Wo = np.asarray(Wo, dtype=np.float32)
    bo = np.asarray(bo, dtype=np.float32)

    xT = np.ascontiguousarray(x.reshape(S, H).T.astype(bf16))
    causal = bool(
        np.array_equal(causal_mask,
                       np.triu(np.ones((S, S), np.float32), k=1)))
    scale = np.float32(1.0 / np.sqrt(D))
    mask_blk = (-1e9 * np.tril(np.ones((128, 128), np.float32), k=-1)
                ).astype(bf16)
    mask_neg = np.full((128, 128), -1e9, dtype=np.float32).astype(bf16)
    ident_b = np.eye(128, dtype=np.float32).astype(bf16)
    ident64 = np.tile(np.eye(64, dtype=np.float32), (2, 1))
    ones64 = np.ones((1, 64), dtype=np.float32).astype(bf16)

    def pack(w, ntiles):
        # [(i p), c] -> [p, i, c] contiguous
        return np.ascontiguousarray(
            w.reshape(ntiles, 128, w.shape[1]).transpose(1, 0, 2))

    in_maps = []
    for g in range(G):
        qsl = slice(g * HDG, (g + 1) * HDG)
        ksl = slice(g * D, (g + 1) * D)
        wq_g = pack((Wq[:, qsl] * scale).astype(bf16), NH)
        bq_g = np.ascontiguousarray(
            (bq[qsl] * scale).reshape(2, 128).T.astype(np.float32))
        wkv_g = pack(
            np.concatenate([Wk[:, ksl], Wv[:, ksl]], axis=1).astype(bf16), NH)
        bkv_g = np.ascontiguousarray(
            np.concatenate([bk[ksl], bv[ksl]]).reshape(128, 1)
            .astype(np.float32))
        wo_g = np.ascontiguousarray(
            Wo[qsl, :].reshape(2, 128, H).transpose(1, 0, 2).astype(bf16))
        m = {
            "xT": xT, "wq": wq_g, "bq": bq_g, "wkv": wkv_g, "bkv": bkv_g,
            "wo": wo_g, "mblk": mask_blk, "identB": ident_b,
            "mneg": mask_neg, "ident64": ident64, "ones64": ones64,
        }
        if not causal:
            m["mfull"] = np.ascontiguousarray(
                (causal_mask.T * np.float32(-1e9)).astype(bf16))
        in_maps.append(m)

    nc = _get_program(causal)
    res = run_bass_kernel_spmd(nc, in_maps, list(range(G)), trace=_trace)
    out = res.results[0]["partial"].astype(np.float32)
    for g in range(1, G):
        out = out + res.results[g]["partial"].astype(np.float32)
    out = out + bo[None, :]
    return out.reshape(1, S, H).astype(np.float32)


if __name__ == "__main__":
    pass
